# revision 20
# baseline (speedup 1.0000x reference)
"""Trainium2 Bass kernel for DistributedAFNO2D (v4).

Problem: x(2,768,256,256) f32; per-block (8 blocks of 96 ch) spectral MLP:
  out = irfft2( softshrink( W2*relu(W1*rfft2(x) + b1) + b2 ) ) + x
Block-diagonal channel mixing with shared-per-(u,v) complex 96x96 weights.

Sharding: block k -> core k (8 cores). No collectives. Each core handles
(2, 96, 256, 256) with its own block weights.

Layouts: x host-reordered to [B, C, 128p, 2j, 256w] (h = 128*j + p) so each
channel moves with ONE dma. zbuf/sbuf_d are [B, C, 128p, 2j, 258] (u = 128*j+p).

v4: all three phases are software-pipelined at depth 1 so the in-order PE
queue never head-of-line blocks on a same-iteration ACT/DVE dependency:
  A: emit [S1(c); S2(c-1)]   B: emit [mix1(g); mix2(g-1)]
  C: emit [psab(c); pso(c-1)]
C(b=0) is interleaved channel-by-channel with A(b=1) (complementary engines).

Per-phase dataflow: see v3 docstring; unchanged math:
  A: S1 contract h via CHpack -> psY [w, u|ri]; S2 contract w via R1/R2
     -> psZ [u, v|ri] -> zbuf (1 dma/channel)
  B: per (j, u-triple): mix1 (4 mm, W stationary, N=G*129) + relu(+b1) ACT;
     mix2 (4 mm); DVE += (b2-lam) -> t' -> sbuf_d; v=0 cols -> dc_sb (gpsimd)
     per (b,j): softshrink dc cols + dma-transpose -> dct [128u, 96c]
  C: psq = DC ifft (8 mm N=96) -> q0sb/16; per channel: softshrink in bf16,
     psab = [QrT|QiT] (4 mm N=512 via [CHIr|CHIi] & [-CHIi|CHIr] packs),
     qrqi copy, pso = irfft_v via Gc/Gs, out = x_bf16 + q0 + pso -> 1 dma
"""
import os
import sys
import numpy as np

sys.path.insert(0, "/opt/trn_rl_repo")

import ml_dtypes

BF16 = ml_dtypes.bfloat16

H = 256
W = 256
NV = W // 2 + 1  # 129
BLK = 96
NCORES = 8
B = 2
LAM = 0.01


def make_host_consts():
    """All packed constant matrices (numpy bf16) via probing np.fft."""
    I = np.eye(H, dtype=np.float64)
    F = np.fft.fft(I, axis=0, norm='ortho')       # F[u,h]; F@x = fft(x)
    Fi = np.fft.ifft(I, axis=0, norm='ortho')     # Fi[h,u]
    CHr = F.real.T.copy()                          # [h,u]
    CHi = F.imag.T.copy()
    EWr = F.real.T[:, :NV].copy()                  # [w,v]
    EWi = F.imag.T[:, :NV].copy()
    CHIr = Fi.real.T.copy()                        # [u,h]
    CHIi = Fi.imag.T.copy()
    Ir = np.eye(NV)
    Gc = np.fft.irfft(Ir, n=W, axis=-1, norm='ortho')        # [v,w]
    Gs = np.fft.irfft(1j * Ir, n=W, axis=-1, norm='ortho')   # [v,w]

    c = {}
    c['chpack'] = np.stack([
        np.concatenate([CHr[j * 128:(j + 1) * 128, :], CHi[j * 128:(j + 1) * 128, :]], axis=1)
        for j in range(2)])
    c['r1'] = np.stack([
        np.concatenate([EWr[j * 128:(j + 1) * 128], EWi[j * 128:(j + 1) * 128]], axis=1)
        for j in range(2)])
    c['r2'] = np.stack([
        np.concatenate([-EWi[j * 128:(j + 1) * 128], EWr[j * 128:(j + 1) * 128]], axis=1)
        for j in range(2)])
    c['chipack'] = np.stack([
        np.concatenate([CHIr[j * 128:(j + 1) * 128], CHIi[j * 128:(j + 1) * 128]], axis=1)
        for j in range(2)])
    c['nchichi'] = np.stack([
        np.concatenate([-CHIi[j * 128:(j + 1) * 128], CHIr[j * 128:(j + 1) * 128]], axis=1)
        for j in range(2)])
    c['nchi'] = np.stack([-CHIi[j * 128:(j + 1) * 128] for j in range(2)])
    c['gc'] = Gc[1:129]
    c['gs'] = Gs[1:129]
    return {k: v.astype(BF16) for k, v in c.items()}


def make_weight_consts(w1k, w2k):
    """w1k/w2k: (96, 96, 2) [i, o, ri]."""
    return {
        'w1r': w1k[..., 0].astype(BF16),
        'w1i': w1k[..., 1].astype(BF16),
        'w1in': (-w1k[..., 1]).astype(BF16),
        'w2r': w2k[..., 0].astype(BF16),
        'w2i': w2k[..., 1].astype(BF16),
        'w2in': (-w2k[..., 1]).astype(BF16),
    }


def build_nc():
    import concourse.bass as bass
    import concourse.tile as tile
    from concourse import bacc, mybir

    dt = mybir.dt
    nc = bacc.Bacc("TRN2", target_bir_lowering=False, debug=False)

    xbf = nc.dram_tensor("xbf", [B, BLK, 128, 2, W], dt.bfloat16, kind="ExternalInput").ap()
    chpack = nc.dram_tensor("chpack", [2, 128, 512], dt.bfloat16, kind="ExternalInput").ap()
    r1 = nc.dram_tensor("r1", [2, 128, 258], dt.bfloat16, kind="ExternalInput").ap()
    r2 = nc.dram_tensor("r2", [2, 128, 258], dt.bfloat16, kind="ExternalInput").ap()
    chipack = nc.dram_tensor("chipack", [2, 128, 512], dt.bfloat16, kind="ExternalInput").ap()
    nchichi = nc.dram_tensor("nchichi", [2, 128, 512], dt.bfloat16, kind="ExternalInput").ap()
    nchi = nc.dram_tensor("nchi", [2, 128, 256], dt.bfloat16, kind="ExternalInput").ap()
    gc = nc.dram_tensor("gc", [128, 256], dt.bfloat16, kind="ExternalInput").ap()
    gs = nc.dram_tensor("gs", [128, 256], dt.bfloat16, kind="ExternalInput").ap()
    wts = {n: nc.dram_tensor(n, [96, 96], dt.bfloat16, kind="ExternalInput").ap()
           for n in ['w1r', 'w1i', 'w1in', 'w2r', 'w2i', 'w2in']}
    b1cols = nc.dram_tensor("b1cols", [96, 2], dt.float32, kind="ExternalInput").ap()
    b2cols = nc.dram_tensor("b2cols", [96, 2], dt.float32, kind="ExternalInput").ap()
    out = nc.dram_tensor("out", [B, BLK, 128, 2, W], dt.float32, kind="ExternalOutput").ap()

    zbuf = nc.dram_tensor("zbuf", [B, BLK, 128, 2, 258], dt.bfloat16).ap()
    sbuf_d = nc.dram_tensor("sbufd", [B, BLK, 128, 2, 258], dt.bfloat16).ap()

    G0 = 3  # u rows per phase-B group
    TLAM = 2.0 * LAM

    with tile.TileContext(nc) as tc:
        from contextlib import ExitStack
        with ExitStack() as ctx:
            consts = ctx.enter_context(tc.tile_pool(name="consts", bufs=1))
            pa = ctx.enter_context(tc.tile_pool(name="pa", bufs=4))
            pb = ctx.enter_context(tc.tile_pool(name="pb", bufs=4))
            pc = ctx.enter_context(tc.tile_pool(name="pc", bufs=5))
            psum = ctx.enter_context(tc.tile_pool(name="psum", bufs=2, space="PSUM"))

            def chunked_const(name, ap_, ncols):
                ts = []
                for j in range(2):
                    t = consts.tile([128, ncols], dt.bfloat16, tag=f"{name}{j}", name=f"{name}{j}")
                    nc.sync.dma_start(out=t, in_=ap_[j])
                    ts.append(t)
                return ts

            t_ch = chunked_const("t_ch", chpack, 512)
            t_r1 = chunked_const("t_r1", r1, 258)
            t_r2 = chunked_const("t_r2", r2, 258)
            t_chi = chunked_const("t_chi", chipack, 512)
            t_ncc = chunked_const("t_ncc", nchichi, 512)
            t_nchi = chunked_const("t_nchi", nchi, 256)
            t_gc = consts.tile([128, 256], dt.bfloat16, tag="t_gc", name="t_gc")
            nc.sync.dma_start(out=t_gc, in_=gc)
            t_gs = consts.tile([128, 256], dt.bfloat16, tag="t_gs", name="t_gs")
            nc.sync.dma_start(out=t_gs, in_=gs)
            t_w = {}
            for n, ap_ in wts.items():
                t_w[n] = consts.tile([96, 96], dt.bfloat16, tag=f"t_{n}", name=f"t_{n}")
                nc.sync.dma_start(out=t_w[n], in_=ap_)

            t_b1 = consts.tile([96, 2], dt.float32, tag="t_b1", name="t_b1")
            nc.sync.dma_start(out=t_b1, in_=b1cols)
            t_b2 = consts.tile([96, 2], dt.float32, tag="t_b2", name="t_b2")
            nc.sync.dma_start(out=t_b2, in_=b2cols)

            # ---------- pipelined phase stages ----------
            def A_load(b, c):
                xt = pa.tile([128, 2, 256], dt.bfloat16, tag="xt", name="xt")
                nc.gpsimd.dma_start(out=xt, in_=xbf[b, c])
                return xt

            def A_s1(b, c, xt):
                """S1 matmuls on prefetched x(c), y copies. Returns A_s2 state."""
                ys = []
                for wc in range(2):
                    psy = psum.tile([128, 512], dt.float32, tag="pA", name="psy")
                    nc.tensor.matmul(psy, lhsT=xt[:, 0, wc * 128:(wc + 1) * 128],
                                     rhs=t_ch[0], start=True, stop=False)
                    nc.tensor.matmul(psy, lhsT=xt[:, 1, wc * 128:(wc + 1) * 128],
                                     rhs=t_ch[1], start=False, stop=True)
                    y = pa.tile([128, 512], dt.bfloat16, tag=f"y{wc}", name=f"y{wc}")
                    if wc == 0:
                        nc.scalar.copy(y, psy)
                    else:
                        nc.vector.tensor_scalar_add(y, psy, 0.0)
                    ys.append(y)
                return (b, c, ys)

            def A_s2(st):
                b, c, ys = st
                zt2 = pa.tile([128, 2, 258], dt.bfloat16, tag="zt2", name="zt2")
                for uc in range(2):
                    psz = psum.tile([128, 258], dt.float32, tag="pB", name="psz")
                    us = slice(uc * 128, (uc + 1) * 128)
                    us2 = slice(256 + uc * 128, 256 + (uc + 1) * 128)
                    nc.tensor.matmul(psz, lhsT=ys[0][:, us], rhs=t_r1[0], start=True, stop=False)
                    nc.tensor.matmul(psz, lhsT=ys[0][:, us2], rhs=t_r2[0], start=False, stop=False)
                    nc.tensor.matmul(psz, lhsT=ys[1][:, us], rhs=t_r1[1], start=False, stop=False)
                    nc.tensor.matmul(psz, lhsT=ys[1][:, us2], rhs=t_r2[1], start=False, stop=True)
                    if uc == 0:
                        nc.vector.tensor_scalar_add(zt2[:, 0, :], psz, 0.0)
                    else:
                        nc.scalar.copy(zt2[:, 1, :], psz)
                nc.sync.dma_start(out=zbuf[b, c], in_=zt2)

            def B_load(b, j, p0, G):
                zt = pb.tile([96, G0, 258], dt.bfloat16, tag="zt", name="zt")
                nc.gpsimd.dma_start(out=zt[:, 0:G, :], in_=zbuf[b, :, p0:p0 + G, j, :])
                return zt

            def B_mix1(zt, G):
                NG = G * 129
                zr = zt[:, 0:G, 0:129]
                zi = zt[:, 0:G, 129:258]
                psR = psum.tile([96, G0 * 129], dt.float32, tag="pA", name="psR")
                psI = psum.tile([96, G0 * 129], dt.float32, tag="pB", name="psI")
                nc.tensor.matmul(psR[:, 0:NG], lhsT=t_w['w1r'], rhs=zr, start=True, stop=False)
                nc.tensor.matmul(psR[:, 0:NG], lhsT=t_w['w1in'], rhs=zi, start=False, stop=True)
                nc.tensor.matmul(psI[:, 0:NG], lhsT=t_w['w1i'], rhs=zr, start=True, stop=False)
                nc.tensor.matmul(psI[:, 0:NG], lhsT=t_w['w1r'], rhs=zi, start=False, stop=True)
                o1r = pb.tile([96, G0 * 129], dt.bfloat16, tag="o1r", name="o1r")
                o1i = pb.tile([96, G0 * 129], dt.bfloat16, tag="o1i", name="o1i")
                nc.scalar.activation(o1r[:, 0:NG], psR[:, 0:NG],
                                     mybir.ActivationFunctionType.Relu, bias=t_b1[:, 0:1])
                nc.scalar.activation(o1i[:, 0:NG], psI[:, 0:NG],
                                     mybir.ActivationFunctionType.Relu, bias=t_b1[:, 1:2])
                return o1r, o1i

            def B_mix2(st):
                b, j, p0, G, o1r, o1i, dc_sb = st
                NG = G * 129
                psR2 = psum.tile([96, G0 * 129], dt.float32, tag="pC", name="psR2")
                psI2 = psum.tile([96, G0 * 129], dt.float32, tag="pD", name="psI2")
                nc.tensor.matmul(psR2[:, 0:NG], lhsT=t_w['w2r'], rhs=o1r[:, 0:NG], start=True, stop=False)
                nc.tensor.matmul(psR2[:, 0:NG], lhsT=t_w['w2in'], rhs=o1i[:, 0:NG], start=False, stop=True)
                nc.tensor.matmul(psI2[:, 0:NG], lhsT=t_w['w2i'], rhs=o1r[:, 0:NG], start=True, stop=False)
                nc.tensor.matmul(psI2[:, 0:NG], lhsT=t_w['w2r'], rhs=o1i[:, 0:NG], start=False, stop=True)
                tr_ = pb.tile([96, G0, 129], dt.bfloat16, tag="tr_", name="tr_")
                ti_ = pb.tile([96, G0, 129], dt.bfloat16, tag="ti_", name="ti_")
                nc.vector.tensor_scalar_add(tr_[:, 0:G, :], psR2[:, 0:NG], t_b2[:, 0:1])
                nc.vector.tensor_scalar_add(ti_[:, 0:G, :], psI2[:, 0:NG], t_b2[:, 1:2])
                nc.sync.dma_start(out=sbuf_d[b, :, p0:p0 + G, j, 0:129], in_=tr_[:, 0:G, :])
                nc.sync.dma_start(out=sbuf_d[b, :, p0:p0 + G, j, 129:258], in_=ti_[:, 0:G, :])
                # collect v=0 cols for the DC term on DVE: they depend on the
                # ts_adds just above, so they block nothing there — on the ACT
                # queue they would delay relu(g+1) until after mix2(g).
                nc.vector.tensor_scalar_add(dc_sb[:, 0, p0:p0 + G], tr_[:, 0:G, 0], 0.0)
                nc.vector.tensor_scalar_add(dc_sb[:, 1, p0:p0 + G], ti_[:, 0:G, 0], 0.0)

            def B_dc_fin(j, dc_sb, dct):
                dccl = pb.tile([96, 2, 128], dt.bfloat16, tag="dccl", name="dccl")
                nc.vector.tensor_scalar(dccl, dc_sb, 0.0, -TLAM,
                                        mybir.AluOpType.min, mybir.AluOpType.max)
                dcs = pb.tile([96, 2, 128], dt.bfloat16, tag="dcs", name="dcs")
                nc.vector.tensor_tensor(dcs, dc_sb, dccl, mybir.AluOpType.subtract)
                for ri in range(2):
                    t = pb.tile([128, 96], dt.bfloat16, tag=f"dct{j}{ri}", name=f"dct{j}{ri}", bufs=2)
                    # sync queue: on ACT it would block the next j's relus
                    nc.sync.dma_start_transpose(out=t, in_=dcs[:, ri, :])
                    dct[(j, ri)] = t

            def phaseB(b):
                dct = {}
                for j in range(2):
                    dc_sb = pb.tile([96, 2, 128], dt.bfloat16, tag="dc_sb",
                                    name="dc_sb", bufs=2)
                    p0 = 0
                    while p0 < 128:
                        G = min(G0, 128 - p0)
                        zt = B_load(b, j, p0, G)
                        o1r, o1i = B_mix1(zt, G)
                        B_mix2((b, j, p0, G, o1r, o1i, dc_sb))
                        p0 += G
                    B_dc_fin(j, dc_sb, dct)
                return dct

            def phaseC_start(b, dct):
                psq = psum.tile([128, 192], dt.float32, tag="pC", name="psq")
                for hc in range(2):
                    hs = slice(hc * 128, (hc + 1) * 128)
                    qs = slice(hc * 96, (hc + 1) * 96)
                    nc.tensor.matmul(psq[:, qs], lhsT=t_chi[0][:, hs], rhs=dct[(0, 0)],
                                     start=True, stop=False, skip_group_check=True)
                    nc.tensor.matmul(psq[:, qs], lhsT=t_nchi[0][:, hs], rhs=dct[(0, 1)],
                                     start=False, stop=False, skip_group_check=True)
                    nc.tensor.matmul(psq[:, qs], lhsT=t_chi[1][:, hs], rhs=dct[(1, 0)],
                                     start=False, stop=False, skip_group_check=True)
                    nc.tensor.matmul(psq[:, qs], lhsT=t_nchi[1][:, hs], rhs=dct[(1, 1)],
                                     start=False, stop=True, skip_group_check=True)
                q0sb = pc.tile([128, 192], dt.float32, tag="q0sb", name="q0sb")
                nc.vector.tensor_scalar_mul(q0sb, psq, 1.0 / 16.0)
                return q0sb

            def C_load(b, c):
                """Prefetch t'(c) and softshrink it (gpsimd + DVE)."""
                st2 = pc.tile([128, 2, 258], dt.bfloat16, tag="st2", name="st2")
                nc.gpsimd.dma_start(out=st2, in_=sbuf_d[b, c])
                cl2 = pc.tile([128, 2, 258], dt.bfloat16, tag="cl2", name="cl2")
                nc.gpsimd.tensor_scalar(cl2, st2, 0.0, -TLAM,
                                        mybir.AluOpType.min, mybir.AluOpType.max)
                s2 = pc.tile([128, 2, 258], dt.bfloat16, tag="s2", name="s2")
                nc.vector.tensor_tensor(s2, st2, cl2, mybir.AluOpType.subtract)
                return s2

            def C_abq(b, c, s2):
                """psab matmuls on prefetched s2, qrqi copy, x residual load."""
                psab = psum.tile([128, 512], dt.float32, tag="pD", name="psab")
                nc.tensor.matmul(psab, lhsT=s2[:, 0, 1:129], rhs=t_chi[0], start=True, stop=False)
                nc.tensor.matmul(psab, lhsT=s2[:, 1, 1:129], rhs=t_chi[1], start=False, stop=False)
                nc.tensor.matmul(psab, lhsT=s2[:, 0, 130:258], rhs=t_ncc[0], start=False, stop=False)
                nc.tensor.matmul(psab, lhsT=s2[:, 1, 130:258], rhs=t_ncc[1], start=False, stop=True)
                qrqi = pc.tile([128, 512], dt.bfloat16, tag="qrqi", name="qrqi")
                nc.scalar.copy(qrqi, psab)
                xt_r = pc.tile([128, 2, 256], dt.bfloat16, tag="xt_r", name="xt_r")
                nc.sync.dma_start(out=xt_r, in_=xbf[b, c])
                return (b, c, qrqi, xt_r)

            def C_out(st, q0sb):
                b, c, qrqi, xt_r = st
                ot = pc.tile([128, 2, 256], dt.float32, tag="ot", name="ot")
                for hc in range(2):
                    pso = psum.tile([128, 256], dt.float32, tag="pC", name="pso")
                    nc.tensor.matmul(pso, lhsT=qrqi[:, hc * 128:(hc + 1) * 128], rhs=t_gc,
                                     start=True, stop=False)
                    nc.tensor.matmul(pso, lhsT=qrqi[:, 256 + hc * 128:256 + (hc + 1) * 128],
                                     rhs=t_gs, start=False, stop=True)
                    nc.vector.scalar_tensor_tensor(
                        ot[:, hc, :], xt_r[:, hc, :], q0sb[:, hc * 96 + c:hc * 96 + c + 1], pso,
                        mybir.AluOpType.add, mybir.AluOpType.add)
                nc.sync.dma_start(out=out[b, c], in_=ot)

            # ---------- schedule (natural order; tile scheduler reorders) ----------
            for c in range(BLK):
                A_s2(A_s1(0, c, A_load(0, c)))

            dct0 = phaseB(0)
            q0sb0 = phaseC_start(0, dct0)

            # C(0) interleaved with A(1): complementary engine mixes
            for c in range(BLK):
                C_out(C_abq(0, c, C_load(0, c)), q0sb0)
                A_s2(A_s1(1, c, A_load(1, c)))

            dct1 = phaseB(1)
            q0sb1 = phaseC_start(1, dct1)

            for c in range(BLK):
                C_out(C_abq(1, c, C_load(1, c)), q0sb1)
    nc.compile()
    return nc


_NC_CACHE = {}


def _get_nc():
    if 'nc' not in _NC_CACHE:
        _NC_CACHE['nc'] = build_nc()
    return _NC_CACHE['nc']


def make_in_maps(x, w1, b1, w2, b2):
    hc = make_host_consts()
    x = np.ascontiguousarray(x, dtype=np.float32)
    in_maps = []
    for k in range(NCORES):
        # [B, 96, 256, 256] -> [B, 96, 128p, 2j, 256w], h = 128*j + p
        xk = x[:, BLK * k:BLK * (k + 1)].reshape(B, BLK, 2, 128, W).transpose(0, 1, 3, 2, 4)
        wk = make_weight_consts(w1[k], w2[k])
        b1k = b1[k, :, 0, 0, :]
        b2k = b2[k, :, 0, 0, :]
        m = dict(
            b1cols=np.ascontiguousarray(b1k, dtype=np.float32),
            b2cols=np.ascontiguousarray(b2k - LAM, dtype=np.float32),
            xbf=np.ascontiguousarray(xk).astype(BF16),
            chpack=hc['chpack'], r1=hc['r1'], r2=hc['r2'],
            chipack=hc['chipack'], nchichi=hc['nchichi'], nchi=hc['nchi'],
            gc=hc['gc'], gs=hc['gs'],
            **wk,
        )
        in_maps.append(m)
    return in_maps


def postprocess(outs):
    """outs: list of [B, 96, 128, 2, 256] per core -> [B, 768, 256, 256]."""
    full = np.concatenate(outs, axis=1)
    return np.ascontiguousarray(
        full.transpose(0, 1, 3, 2, 4).reshape(B, BLK * NCORES, H, W))


def kernel(x, w1, b1, w2, b2):
    from concourse.bass_utils import run_bass_kernel_spmd
    nc = _get_nc()
    in_maps = make_in_maps(np.asarray(x), np.asarray(w1), np.asarray(b1),
                           np.asarray(w2), np.asarray(b2))
    res = run_bass_kernel_spmd(nc, in_maps, core_ids=list(range(NCORES)))
    return postprocess([res.results[k]['out'] for k in range(NCORES)])


# revision 21
# speedup vs baseline: 1.3037x; 1.3037x over previous
"""Trainium2 Bass kernel for DistributedAFNO2D (v4).

Problem: x(2,768,256,256) f32; per-block (8 blocks of 96 ch) spectral MLP:
  out = irfft2( softshrink( W2*relu(W1*rfft2(x) + b1) + b2 ) ) + x
Block-diagonal channel mixing with shared-per-(u,v) complex 96x96 weights.

Sharding: block k -> core k (8 cores). No collectives. Each core handles
(2, 96, 256, 256) with its own block weights.

Layouts: x host-reordered to [B, C, 128p, 2j, 256w] (h = 128*j + p) so each
channel moves with ONE dma. zbuf/sbuf_d are [B, C, 128p, 2j, 258] (u = 128*j+p).

v4: all three phases are software-pipelined at depth 1 so the in-order PE
queue never head-of-line blocks on a same-iteration ACT/DVE dependency:
  A: emit [S1(c); S2(c-1)]   B: emit [mix1(g); mix2(g-1)]
  C: emit [psab(c); pso(c-1)]
C(b=0) is interleaved channel-by-channel with A(b=1) (complementary engines).

Per-phase dataflow: see v3 docstring; unchanged math:
  A: S1 contract h via CHpack -> psY [w, u|ri]; S2 contract w via R1/R2
     -> psZ [u, v|ri] -> zbuf (1 dma/channel)
  B: per (j, u-triple): mix1 (4 mm, W stationary, N=G*129) + relu(+b1) ACT;
     mix2 (4 mm); DVE += (b2-lam) -> t' -> sbuf_d; v=0 cols -> dc_sb (gpsimd)
     per (b,j): softshrink dc cols + dma-transpose -> dct [128u, 96c]
  C: psq = DC ifft (8 mm N=96) -> q0sb/16; per channel: softshrink in bf16,
     psab = [QrT|QiT] (4 mm N=512 via [CHIr|CHIi] & [-CHIi|CHIr] packs),
     qrqi copy, pso = irfft_v via Gc/Gs, out = x_bf16 + q0 + pso -> 1 dma
"""
import os
import sys
import numpy as np

sys.path.insert(0, "/opt/trn_rl_repo")

import ml_dtypes

BF16 = ml_dtypes.bfloat16

H = 256
W = 256
NV = W // 2 + 1  # 129
BLK = 96
NCORES = 8
B = 2
LAM = 0.01


def make_host_consts():
    """All packed constant matrices (numpy bf16) via probing np.fft."""
    I = np.eye(H, dtype=np.float64)
    F = np.fft.fft(I, axis=0, norm='ortho')       # F[u,h]; F@x = fft(x)
    Fi = np.fft.ifft(I, axis=0, norm='ortho')     # Fi[h,u]
    CHr = F.real.T.copy()                          # [h,u]
    CHi = F.imag.T.copy()
    EWr = F.real.T[:, :NV].copy()                  # [w,v]
    EWi = F.imag.T[:, :NV].copy()
    CHIr = Fi.real.T.copy()                        # [u,h]
    CHIi = Fi.imag.T.copy()
    Ir = np.eye(NV)
    Gc = np.fft.irfft(Ir, n=W, axis=-1, norm='ortho')        # [v,w]
    Gs = np.fft.irfft(1j * Ir, n=W, axis=-1, norm='ortho')   # [v,w]

    c = {}
    c['chpack'] = np.stack([
        np.concatenate([CHr[j * 128:(j + 1) * 128, :], CHi[j * 128:(j + 1) * 128, :]], axis=1)
        for j in range(2)])
    c['r1'] = np.stack([
        np.concatenate([EWr[j * 128:(j + 1) * 128], EWi[j * 128:(j + 1) * 128]], axis=1)
        for j in range(2)])
    c['r2'] = np.stack([
        np.concatenate([-EWi[j * 128:(j + 1) * 128], EWr[j * 128:(j + 1) * 128]], axis=1)
        for j in range(2)])
    c['chipack'] = np.stack([
        np.concatenate([CHIr[j * 128:(j + 1) * 128], CHIi[j * 128:(j + 1) * 128]], axis=1)
        for j in range(2)])
    c['nchichi'] = np.stack([
        np.concatenate([-CHIi[j * 128:(j + 1) * 128], CHIr[j * 128:(j + 1) * 128]], axis=1)
        for j in range(2)])
    c['nchi'] = np.stack([-CHIi[j * 128:(j + 1) * 128] for j in range(2)])
    c['gc'] = Gc[1:129]
    c['gs'] = Gs[1:129]
    return {k: v.astype(BF16) for k, v in c.items()}


def make_weight_consts(w1k, w2k):
    """w1k/w2k: (96, 96, 2) [i, o, ri]."""
    return {
        'w1r': w1k[..., 0].astype(BF16),
        'w1i': w1k[..., 1].astype(BF16),
        'w1in': (-w1k[..., 1]).astype(BF16),
        'w2r': w2k[..., 0].astype(BF16),
        'w2i': w2k[..., 1].astype(BF16),
        'w2in': (-w2k[..., 1]).astype(BF16),
    }


def build_nc():
    import concourse.bass as bass
    import concourse.tile as tile
    from concourse import bacc, mybir

    dt = mybir.dt
    nc = bacc.Bacc("TRN2", target_bir_lowering=False, debug=False)

    xbf = nc.dram_tensor("xbf", [B, BLK, 128, 2, W], dt.bfloat16, kind="ExternalInput").ap()
    chpack = nc.dram_tensor("chpack", [2, 128, 512], dt.bfloat16, kind="ExternalInput").ap()
    r1 = nc.dram_tensor("r1", [2, 128, 258], dt.bfloat16, kind="ExternalInput").ap()
    r2 = nc.dram_tensor("r2", [2, 128, 258], dt.bfloat16, kind="ExternalInput").ap()
    chipack = nc.dram_tensor("chipack", [2, 128, 512], dt.bfloat16, kind="ExternalInput").ap()
    nchichi = nc.dram_tensor("nchichi", [2, 128, 512], dt.bfloat16, kind="ExternalInput").ap()
    nchi = nc.dram_tensor("nchi", [2, 128, 256], dt.bfloat16, kind="ExternalInput").ap()
    gc = nc.dram_tensor("gc", [128, 256], dt.bfloat16, kind="ExternalInput").ap()
    gs = nc.dram_tensor("gs", [128, 256], dt.bfloat16, kind="ExternalInput").ap()
    wts = {n: nc.dram_tensor(n, [96, 96], dt.bfloat16, kind="ExternalInput").ap()
           for n in ['w1r', 'w1i', 'w1in', 'w2r', 'w2i', 'w2in']}
    b1cols = nc.dram_tensor("b1cols", [96, 2], dt.float32, kind="ExternalInput").ap()
    b2cols = nc.dram_tensor("b2cols", [96, 2], dt.float32, kind="ExternalInput").ap()
    out = nc.dram_tensor("out", [B, BLK, 128, 2, W], dt.float32, kind="ExternalOutput").ap()

    zbuf = nc.dram_tensor("zbuf", [B, BLK, 128, 2, 258], dt.bfloat16).ap()
    sbuf_d = nc.dram_tensor("sbufd", [B, BLK, 128, 2, 258], dt.bfloat16).ap()

    G0 = 3  # u rows per phase-B group
    TLAM = 2.0 * LAM

    with tile.TileContext(nc) as tc:
        from contextlib import ExitStack
        with ExitStack() as ctx:
            consts = ctx.enter_context(tc.tile_pool(name="consts", bufs=1))
            pa = ctx.enter_context(tc.tile_pool(name="pa", bufs=4))
            pb = ctx.enter_context(tc.tile_pool(name="pb", bufs=4))
            pc = ctx.enter_context(tc.tile_pool(name="pc", bufs=4))
            psum = ctx.enter_context(tc.tile_pool(name="psum", bufs=2, space="PSUM"))

            def chunked_const(name, ap_, ncols):
                ts = []
                for j in range(2):
                    t = consts.tile([128, ncols], dt.bfloat16, tag=f"{name}{j}", name=f"{name}{j}")
                    nc.sync.dma_start(out=t, in_=ap_[j])
                    ts.append(t)
                return ts

            t_ch = chunked_const("t_ch", chpack, 512)
            t_r1 = chunked_const("t_r1", r1, 258)
            t_r2 = chunked_const("t_r2", r2, 258)
            t_chi = chunked_const("t_chi", chipack, 512)
            t_ncc = chunked_const("t_ncc", nchichi, 512)
            t_nchi = chunked_const("t_nchi", nchi, 256)
            t_gc = consts.tile([128, 256], dt.bfloat16, tag="t_gc", name="t_gc")
            nc.sync.dma_start(out=t_gc, in_=gc)
            t_gs = consts.tile([128, 256], dt.bfloat16, tag="t_gs", name="t_gs")
            nc.sync.dma_start(out=t_gs, in_=gs)
            t_w = {}
            for n, ap_ in wts.items():
                t_w[n] = consts.tile([96, 96], dt.bfloat16, tag=f"t_{n}", name=f"t_{n}")
                nc.sync.dma_start(out=t_w[n], in_=ap_)

            t_b1 = consts.tile([96, 2], dt.float32, tag="t_b1", name="t_b1")
            nc.sync.dma_start(out=t_b1, in_=b1cols)
            t_b2 = consts.tile([96, 2], dt.float32, tag="t_b2", name="t_b2")
            nc.sync.dma_start(out=t_b2, in_=b2cols)

            # ---------- pipelined phase stages ----------
            def A_load(b, c):
                xt = pa.tile([128, 2, 256], dt.bfloat16, tag="xt", name="xt")
                nc.gpsimd.dma_start(out=xt, in_=xbf[b, c])
                return xt

            def A_s1(b, c, xt):
                """S1 matmuls on prefetched x(c), y copies. Returns A_s2 state."""
                ys = []
                for wc in range(2):
                    psy = psum.tile([128, 512], dt.float32, tag="pA", name="psy")
                    nc.tensor.matmul(psy, lhsT=xt[:, 0, wc * 128:(wc + 1) * 128],
                                     rhs=t_ch[0], start=True, stop=False)
                    nc.tensor.matmul(psy, lhsT=xt[:, 1, wc * 128:(wc + 1) * 128],
                                     rhs=t_ch[1], start=False, stop=True)
                    y = pa.tile([128, 512], dt.bfloat16, tag=f"y{wc}", name=f"y{wc}")
                    if wc == 0:
                        nc.scalar.copy(y, psy)
                    else:
                        nc.vector.tensor_scalar_add(y, psy, 0.0)
                    ys.append(y)
                return (b, c, ys)

            def A_s2(st):
                b, c, ys = st
                zt2 = pa.tile([128, 2, 258], dt.bfloat16, tag="zt2", name="zt2")
                for uc in range(2):
                    psz = psum.tile([128, 258], dt.float32, tag="pB", name="psz")
                    us = slice(uc * 128, (uc + 1) * 128)
                    us2 = slice(256 + uc * 128, 256 + (uc + 1) * 128)
                    nc.tensor.matmul(psz, lhsT=ys[0][:, us], rhs=t_r1[0], start=True, stop=False)
                    nc.tensor.matmul(psz, lhsT=ys[0][:, us2], rhs=t_r2[0], start=False, stop=False)
                    nc.tensor.matmul(psz, lhsT=ys[1][:, us], rhs=t_r1[1], start=False, stop=False)
                    nc.tensor.matmul(psz, lhsT=ys[1][:, us2], rhs=t_r2[1], start=False, stop=True)
                    if uc == 0:
                        nc.vector.tensor_scalar_add(zt2[:, 0, :], psz, 0.0)
                    else:
                        nc.scalar.copy(zt2[:, 1, :], psz)
                nc.sync.dma_start(out=zbuf[b, c], in_=zt2)

            def B_load(b, j, p0, G):
                zt = pb.tile([96, G0, 258], dt.bfloat16, tag="zt", name="zt")
                nc.gpsimd.dma_start(out=zt[:, 0:G, :], in_=zbuf[b, :, p0:p0 + G, j, :])
                return zt

            def B_mix1(zt, G):
                NG = G * 129
                zr = zt[:, 0:G, 0:129]
                zi = zt[:, 0:G, 129:258]
                psR = psum.tile([96, G0 * 129], dt.float32, tag="pA", name="psR")
                psI = psum.tile([96, G0 * 129], dt.float32, tag="pB", name="psI")
                nc.tensor.matmul(psR[:, 0:NG], lhsT=t_w['w1r'], rhs=zr, start=True, stop=False)
                nc.tensor.matmul(psR[:, 0:NG], lhsT=t_w['w1in'], rhs=zi, start=False, stop=True)
                nc.tensor.matmul(psI[:, 0:NG], lhsT=t_w['w1i'], rhs=zr, start=True, stop=False)
                nc.tensor.matmul(psI[:, 0:NG], lhsT=t_w['w1r'], rhs=zi, start=False, stop=True)
                o1r = pb.tile([96, G0 * 129], dt.bfloat16, tag="o1r", name="o1r")
                o1i = pb.tile([96, G0 * 129], dt.bfloat16, tag="o1i", name="o1i")
                nc.scalar.activation(o1r[:, 0:NG], psR[:, 0:NG],
                                     mybir.ActivationFunctionType.Relu, bias=t_b1[:, 0:1])
                nc.scalar.activation(o1i[:, 0:NG], psI[:, 0:NG],
                                     mybir.ActivationFunctionType.Relu, bias=t_b1[:, 1:2])
                return o1r, o1i

            def B_mix2(st):
                b, j, p0, G, o1r, o1i, dc_sb = st
                NG = G * 129
                psR2 = psum.tile([96, G0 * 129], dt.float32, tag="pC", name="psR2")
                psI2 = psum.tile([96, G0 * 129], dt.float32, tag="pD", name="psI2")
                nc.tensor.matmul(psR2[:, 0:NG], lhsT=t_w['w2r'], rhs=o1r[:, 0:NG], start=True, stop=False)
                nc.tensor.matmul(psR2[:, 0:NG], lhsT=t_w['w2in'], rhs=o1i[:, 0:NG], start=False, stop=True)
                nc.tensor.matmul(psI2[:, 0:NG], lhsT=t_w['w2i'], rhs=o1r[:, 0:NG], start=True, stop=False)
                nc.tensor.matmul(psI2[:, 0:NG], lhsT=t_w['w2r'], rhs=o1i[:, 0:NG], start=False, stop=True)
                tr_ = pb.tile([96, G0, 129], dt.bfloat16, tag="tr_", name="tr_")
                ti_ = pb.tile([96, G0, 129], dt.bfloat16, tag="ti_", name="ti_")
                nc.vector.tensor_scalar_add(tr_[:, 0:G, :], psR2[:, 0:NG], t_b2[:, 0:1])
                nc.vector.tensor_scalar_add(ti_[:, 0:G, :], psI2[:, 0:NG], t_b2[:, 1:2])
                nc.sync.dma_start(out=sbuf_d[b, :, p0:p0 + G, j, 0:129], in_=tr_[:, 0:G, :])
                nc.sync.dma_start(out=sbuf_d[b, :, p0:p0 + G, j, 129:258], in_=ti_[:, 0:G, :])
                # collect v=0 cols for the DC term on DVE: they depend on the
                # ts_adds just above, so they block nothing there — on the ACT
                # queue they would delay relu(g+1) until after mix2(g).
                nc.vector.tensor_scalar_add(dc_sb[:, 0, p0:p0 + G], tr_[:, 0:G, 0], 0.0)
                nc.vector.tensor_scalar_add(dc_sb[:, 1, p0:p0 + G], ti_[:, 0:G, 0], 0.0)

            def B_dc_fin(j, dc_sb, dct):
                dccl = pb.tile([96, 2, 128], dt.bfloat16, tag="dccl", name="dccl")
                nc.vector.tensor_scalar(dccl, dc_sb, 0.0, -TLAM,
                                        mybir.AluOpType.min, mybir.AluOpType.max)
                dcs = pb.tile([96, 2, 128], dt.bfloat16, tag="dcs", name="dcs")
                nc.vector.tensor_tensor(dcs, dc_sb, dccl, mybir.AluOpType.subtract)
                for ri in range(2):
                    t = pb.tile([128, 96], dt.bfloat16, tag=f"dct{j}{ri}", name=f"dct{j}{ri}", bufs=2)
                    # sync queue: on ACT it would block the next j's relus
                    nc.sync.dma_start_transpose(out=t, in_=dcs[:, ri, :])
                    dct[(j, ri)] = t

            def phaseB(b):
                dct = {}
                for j in range(2):
                    dc_sb = pb.tile([96, 2, 128], dt.bfloat16, tag="dc_sb",
                                    name="dc_sb", bufs=2)
                    p0 = 0
                    while p0 < 128:
                        G = min(G0, 128 - p0)
                        zt = B_load(b, j, p0, G)
                        o1r, o1i = B_mix1(zt, G)
                        B_mix2((b, j, p0, G, o1r, o1i, dc_sb))
                        p0 += G
                    B_dc_fin(j, dc_sb, dct)
                return dct

            def phaseC_start(b, dct):
                psq = psum.tile([128, 192], dt.float32, tag="pC", name="psq")
                for hc in range(2):
                    hs = slice(hc * 128, (hc + 1) * 128)
                    qs = slice(hc * 96, (hc + 1) * 96)
                    nc.tensor.matmul(psq[:, qs], lhsT=t_chi[0][:, hs], rhs=dct[(0, 0)],
                                     start=True, stop=False, skip_group_check=True)
                    nc.tensor.matmul(psq[:, qs], lhsT=t_nchi[0][:, hs], rhs=dct[(0, 1)],
                                     start=False, stop=False, skip_group_check=True)
                    nc.tensor.matmul(psq[:, qs], lhsT=t_chi[1][:, hs], rhs=dct[(1, 0)],
                                     start=False, stop=False, skip_group_check=True)
                    nc.tensor.matmul(psq[:, qs], lhsT=t_nchi[1][:, hs], rhs=dct[(1, 1)],
                                     start=False, stop=True, skip_group_check=True)
                q0sb = pc.tile([128, 192], dt.float32, tag="q0sb", name="q0sb")
                nc.vector.tensor_scalar_mul(q0sb, psq, 1.0 / 16.0)
                return q0sb

            def C_load(b, c):
                """Prefetch t'(c) and softshrink it (gpsimd + DVE)."""
                st2 = pc.tile([128, 2, 258], dt.bfloat16, tag="st2", name="st2")
                nc.gpsimd.dma_start(out=st2, in_=sbuf_d[b, c])
                cl2 = pc.tile([128, 2, 258], dt.bfloat16, tag="cl2", name="cl2")
                nc.gpsimd.tensor_scalar(cl2, st2, 0.0, -TLAM,
                                        mybir.AluOpType.min, mybir.AluOpType.max)
                s2 = pc.tile([128, 2, 258], dt.bfloat16, tag="s2", name="s2")
                nc.vector.tensor_tensor(s2, st2, cl2, mybir.AluOpType.subtract)
                return s2

            def C_abq(b, c, s2):
                """psab matmuls on prefetched s2, qrqi copy, x residual load."""
                psab = psum.tile([128, 512], dt.float32, tag="pD", name="psab")
                nc.tensor.matmul(psab, lhsT=s2[:, 0, 1:129], rhs=t_chi[0], start=True, stop=False)
                nc.tensor.matmul(psab, lhsT=s2[:, 1, 1:129], rhs=t_chi[1], start=False, stop=False)
                nc.tensor.matmul(psab, lhsT=s2[:, 0, 130:258], rhs=t_ncc[0], start=False, stop=False)
                nc.tensor.matmul(psab, lhsT=s2[:, 1, 130:258], rhs=t_ncc[1], start=False, stop=True)
                qrqi = pc.tile([128, 512], dt.bfloat16, tag="qrqi", name="qrqi")
                nc.scalar.copy(qrqi, psab)
                xt_r = pc.tile([128, 2, 256], dt.bfloat16, tag="xt_r", name="xt_r")
                nc.sync.dma_start(out=xt_r, in_=xbf[b, c])
                return (b, c, qrqi, xt_r)

            def C_out(st, q0sb):
                b, c, qrqi, xt_r = st
                ot = pc.tile([128, 2, 256], dt.float32, tag="ot", name="ot")
                for hc in range(2):
                    pso = psum.tile([128, 256], dt.float32, tag="pC", name="pso")
                    nc.tensor.matmul(pso, lhsT=qrqi[:, hc * 128:(hc + 1) * 128], rhs=t_gc,
                                     start=True, stop=False)
                    nc.tensor.matmul(pso, lhsT=qrqi[:, 256 + hc * 128:256 + (hc + 1) * 128],
                                     rhs=t_gs, start=False, stop=True)
                    nc.vector.scalar_tensor_tensor(
                        ot[:, hc, :], xt_r[:, hc, :], q0sb[:, hc * 96 + c:hc * 96 + c + 1], pso,
                        mybir.AluOpType.add, mybir.AluOpType.add)
                nc.sync.dma_start(out=out[b, c], in_=ot)

            # ---------- schedule (natural order; tile scheduler reorders) ----------
            for c in range(BLK):
                A_s2(A_s1(0, c, A_load(0, c)))

            dct0 = phaseB(0)
            q0sb0 = phaseC_start(0, dct0)

            # C(0) interleaved with A(1): complementary engine mixes
            for c in range(BLK):
                C_out(C_abq(0, c, C_load(0, c)), q0sb0)
                A_s2(A_s1(1, c, A_load(1, c)))

            dct1 = phaseB(1)
            q0sb1 = phaseC_start(1, dct1)

            for c in range(BLK):
                C_out(C_abq(1, c, C_load(1, c)), q0sb1)
    nc.compile()
    return nc


_NC_CACHE = {}


def _get_nc():
    if 'nc' not in _NC_CACHE:
        _NC_CACHE['nc'] = build_nc()
    return _NC_CACHE['nc']


def make_in_maps(x, w1, b1, w2, b2):
    hc = make_host_consts()
    x = np.ascontiguousarray(x, dtype=np.float32)
    in_maps = []
    for k in range(NCORES):
        # [B, 96, 256, 256] -> [B, 96, 128p, 2j, 256w], h = 128*j + p
        xk = x[:, BLK * k:BLK * (k + 1)].reshape(B, BLK, 2, 128, W).transpose(0, 1, 3, 2, 4)
        wk = make_weight_consts(w1[k], w2[k])
        b1k = b1[k, :, 0, 0, :]
        b2k = b2[k, :, 0, 0, :]
        m = dict(
            b1cols=np.ascontiguousarray(b1k, dtype=np.float32),
            b2cols=np.ascontiguousarray(b2k - LAM, dtype=np.float32),
            xbf=np.ascontiguousarray(xk).astype(BF16),
            chpack=hc['chpack'], r1=hc['r1'], r2=hc['r2'],
            chipack=hc['chipack'], nchichi=hc['nchichi'], nchi=hc['nchi'],
            gc=hc['gc'], gs=hc['gs'],
            **wk,
        )
        in_maps.append(m)
    return in_maps


def postprocess(outs):
    """outs: list of [B, 96, 128, 2, 256] per core -> [B, 768, 256, 256]."""
    full = np.concatenate(outs, axis=1)
    return np.ascontiguousarray(
        full.transpose(0, 1, 3, 2, 4).reshape(B, BLK * NCORES, H, W))


def kernel(x, w1, b1, w2, b2):
    from concourse.bass_utils import run_bass_kernel_spmd
    nc = _get_nc()
    in_maps = make_in_maps(np.asarray(x), np.asarray(w1), np.asarray(b1),
                           np.asarray(w2), np.asarray(b2))
    res = run_bass_kernel_spmd(nc, in_maps, core_ids=list(range(NCORES)))
    return postprocess([res.results[k]['out'] for k in range(NCORES)])
